# revision 19
# baseline (speedup 1.0000x reference)
"""Trainium2 Bass kernel for nn_Decoder_30777735643309.

GRU decoder: ses = tanh(lin1(ses_encoding)); 50 sequential GRU steps with
hidden input concat(h, ses); per-step logits over a 10004 vocab.

Strategy (8 cores, no collectives): data-parallel over batch (16 rows/core),
transposed on-chip layout (features on partitions, batch/time in the free
dim).  Key structure:
  1. Hfull = [h, ses] with ses constant -> gh = h @ Whh[:, :H].T + CT where
     CT = ses @ Whh[:, H:].T + bhh is computed once.
  2. Critical-path split: only the first H columns of each gate (r,z,n) feed
     the recurrent state h' = hnew[:, :H].  Gate rows are permuted on the
     host so those 12 chunks are contiguous; only they are computed inside
     the sequential loop.  The other 12 chunks (needed only for
     hnew[:, H:] -> logits) are recomputed afterwards as batched matmuls
     over all 50 steps.
  3. The in-loop recurrent weights are fp8 (e3m4) so the per-step LDWEIGHTS
     stream through the PE array is 4x narrower; activations stay bf16 and
     accumulation is fp32.
  4. All batched work (embedding gather, gx precompute, deferred gates,
     output projection, logits) is interleaved into the tensor-engine gaps
     of the sequential recurrence; SBUF-only elementwise work runs on the
     otherwise-idle GPSIMD engine.
Logits are written bf16 and upcast on the host.
"""

import numpy as np
import ml_dtypes

import concourse.bacc as bacc
import concourse.mybir as mybir
import concourse.tile as tile
from concourse.bass import IndirectOffsetOnAxis
from concourse.bass_utils import run_bass_kernel_spmd
from concourse.masks import make_identity

F32 = mybir.dt.float32
BF16 = mybir.dt.bfloat16
FP8 = mybir.dt.float8e3
I32 = mybir.dt.int32
AF = mybir.ActivationFunctionType
OP = mybir.AluOpType

V = 10004
E = 300
EP = 384          # E padded to 3 K-chunks of 128
SH = 1024
H = 512
G = 1024          # GRU hidden = 2*H
G3 = 3 * G        # 3072
B, T = 128, 50
NCORES = 8
BL = B // NCORES  # 16 batch rows per core
NT = T * BL       # 800 (t-major columns: col = t*BL + b)
NTP = 896         # NT padded to 7 chunks of 128 (DRAM out rows)
KH = H // 128     # 4 K-chunks for the h-part matmul
M3 = G3 // 128    # 24 feature chunks of the gate dim
MC = 12           # critical chunks (r_c, z_c, n_c)
MCW = MC * 128    # 1536
NB = 2            # column blocking for the batched matmuls
NBW = NT // NB    # 400 columns per block
NV = 20           # vocab blocks of 512

# permutation of the 3G gate dim: crit-first
PERM = np.r_[0:H, G:G + H, 2 * G:2 * G + H,
             H:G, G + H:2 * G, 2 * G + H:3 * G]


def build_program(reps: int = 1, debug: bool = False, loop: bool = False, variant: str = "full"):
    nc = bacc.Bacc()

    # ---- DRAM I/O ----
    d_sesenc = nc.dram_tensor("sesenc", [SH, BL], BF16, kind="ExternalInput")
    d_xw = nc.dram_tensor("xw", [128, 7], I32, kind="ExternalInput")
    d_emb = nc.dram_tensor("emb", [V, E], F32, kind="ExternalInput")
    d_whh8 = nc.dram_tensor("whh8", [H, MCW], FP8, kind="ExternalInput")
    d_whh8b = nc.dram_tensor("whh8b", [H, MCW], BF16, kind="ExternalInput")
    d_whh_n = nc.dram_tensor("whh_n", [H, G3 - MCW], BF16, kind="ExternalInput")
    d_whh_s = nc.dram_tensor("whh_s", [H, G3], BF16, kind="ExternalInput")
    d_wih = nc.dram_tensor("wih", [EP, G3], BF16, kind="ExternalInput")
    d_w1 = nc.dram_tensor("w1", [SH, H], BF16, kind="ExternalInput")
    d_w2 = nc.dram_tensor("w2", [G, E], BF16, kind="ExternalInput")
    d_wout = nc.dram_tensor("wout", [EP, V], BF16, kind="ExternalInput")
    d_b1t = nc.dram_tensor("b1t", [128, H // 128], F32, kind="ExternalInput")
    d_biht = nc.dram_tensor("biht", [128, M3], F32, kind="ExternalInput")
    d_bibht = nc.dram_tensor("bibht", [128, M3], F32, kind="ExternalInput")
    d_bhht = nc.dram_tensor("bhht", [128, M3], F32, kind="ExternalInput")
    d_b2t = nc.dram_tensor("b2t", [128, EP // 128], F32, kind="ExternalInput")
    # t-major rows (row = t*BL + b); rows NT..NTP are junk; host reorders
    d_out = nc.dram_tensor("out", [NTP, V], BF16, kind="ExternalOutput")

    with tile.TileContext(nc) as tc:
        import contextlib
        with contextlib.ExitStack() as ctx:
            persist = ctx.enter_context(tc.tile_pool(name="persist", bufs=1))
            step = ctx.enter_context(tc.tile_pool(name="step", bufs=2))
            post = ctx.enter_context(tc.tile_pool(name="post", bufs=2))
            inp = ctx.enter_context(tc.tile_pool(name="inp", bufs=2))
            gatherp = ctx.enter_context(tc.tile_pool(name="gatherp", bufs=3))
            woutp = ctx.enter_context(tc.tile_pool(name="woutp", bufs=3))
            lout = ctx.enter_context(tc.tile_pool(name="lout", bufs=2))
            psG = ctx.enter_context(tc.tile_pool(name="psG", bufs=1, space="PSUM"))
            psMM = ctx.enter_context(tc.tile_pool(name="psMM", bufs=3, space="PSUM"))
            psL = ctx.enter_context(tc.tile_pool(name="psL", bufs=2, space="PSUM"))
            psT = ctx.enter_context(tc.tile_pool(name="psT", bufs=1, space="PSUM"))

            # persistent SBUF tensors (weights loaded once, outside rep loop)
            whh8_sb = persist.tile([128, KH, MCW], FP8)
            whh8b_sb = persist.tile([128, KH, MCW], BF16)
            whh_n_sb = persist.tile([128, KH, G3 - MCW], BF16)
            whh_s_sb = persist.tile([128, KH, G3], BF16)
            wih_sb = persist.tile([128, EP // 128, G3], BF16)
            w1_sb = persist.tile([128, SH // 128, H], BF16)
            w2_sb = persist.tile([128, G // 128, E], BF16)
            b1t = persist.tile([128, H // 128], F32)
            biht = persist.tile([128, M3], F32)
            bibht = persist.tile([128, M3], F32)
            bhht = persist.tile([128, M3], F32)
            b2t = persist.tile([128, EP // 128], F32)
            ident = persist.tile([128, 128], F32)
            # per-rep working tensors
            gxc = persist.tile([128, MC, NT], BF16)
            gxn2 = persist.tile([128, 4, NT], BF16)
            embxT = persist.tile([128, EP // 128, NT], BF16)
            hsT = persist.tile([128, KH, NT + 2 * BL], BF16)  # states s_0..s_50
            hnT_nc = persist.tile([128, KH, NT], BF16)
            oT = persist.tile([128, EP // 128, NT], BF16)
            ct = persist.tile([128, 4, BL], F32)
            sesT_bf = persist.tile([128, KH, BL], BF16)
            ses_rep = persist.tile([128, KH, NBW], BF16)

            nc.sync.dma_start(out=whh8_sb, in_=d_whh8[:, :].rearrange("(k p) c -> p k c", p=128))
            nc.sync.dma_start(out=whh8b_sb, in_=d_whh8b[:, :].rearrange("(k p) c -> p k c", p=128))
            nc.sync.dma_start(out=whh_n_sb, in_=d_whh_n[:, :].rearrange("(k p) c -> p k c", p=128))
            nc.sync.dma_start(out=whh_s_sb, in_=d_whh_s[:, :].rearrange("(k p) c -> p k c", p=128))
            nc.sync.dma_start(out=wih_sb, in_=d_wih[:, :].rearrange("(k p) c -> p k c", p=128))
            nc.sync.dma_start(out=w1_sb, in_=d_w1[:, :].rearrange("(k p) c -> p k c", p=128))
            nc.sync.dma_start(out=w2_sb, in_=d_w2[:, :].rearrange("(k p) c -> p k c", p=128))
            nc.sync.dma_start(out=b1t, in_=d_b1t[:, :])
            nc.sync.dma_start(out=biht, in_=d_biht[:, :])
            nc.sync.dma_start(out=bibht, in_=d_bibht[:, :])
            nc.sync.dma_start(out=bhht, in_=d_bhht[:, :])
            nc.sync.dma_start(out=b2t, in_=d_b2t[:, :])
            make_identity(nc, ident)
            nc.vector.memset(embxT[:, 2, :], 0.0)
            nc.vector.memset(oT[:, 2, :], 0.0)

            import contextlib as _ctxlib

            if loop:
                loop_cm = tc.For_i(0, reps, 1)
                rep_iter = [0]
            else:
                loop_cm = _ctxlib.nullcontext()
                rep_iter = range(reps)

            # ---------- emission helpers (each call emits one work unit) ----
            def u_gather(c):
                pm = 128 if c < 6 else NT - 6 * 128
                xw = xw_holder[0]
                embx_c = gatherp.tile([128, E], F32, tag="gx")
                nc.gpsimd.indirect_dma_start(
                    out=embx_c[:pm, :], out_offset=None,
                    in_=d_emb[:, :],
                    in_offset=IndirectOffsetOnAxis(ap=xw[:pm, c:c + 1], axis=0))
                for k in range(EP // 128):
                    kw = min(128, E - k * 128)
                    if kw <= 0:
                        break
                    cw = min(128, NT - c * 128)
                    ps_t = psMM.tile([128, 128], F32, tag="mm")
                    nc.tensor.transpose(
                        out=ps_t[:kw, :pm],
                        in_=embx_c[:pm, k * 128:k * 128 + kw],
                        identity=ident[:pm, :pm])
                    nc.scalar.copy(
                        embxT[:kw, k, c * 128:c * 128 + cw], ps_t[:kw, :cw])

            def u_gx(mi, m, nb):
                # crit gx chunk (stores into gxc); rz chunks fold the per-batch
                # ct = ses@Whh_s.T via extra matmuls against ses_rep so the
                # PSUM evac is a single ScalarE op (keeps DVE free for the
                # recurrence chain); biases ride the activation bias port.
                cs = slice(nb * NBW, (nb + 1) * NBW)
                ps_gx = psMM.tile([128, NBW], F32, tag="mm")
                for k in range(EP // 128):
                    nc.tensor.matmul(
                        out=ps_gx,
                        lhsT=wih_sb[:, k, m * 128:(m + 1) * 128],
                        rhs=embxT[:, k, cs],
                        start=(k == 0),
                        stop=(m >= 8 and k == EP // 128 - 1))
                if m < 8:
                    for k in range(KH):
                        nc.tensor.matmul(
                            out=ps_gx,
                            lhsT=whh_s_sb[:, k, m * 128:(m + 1) * 128],
                            rhs=ses_rep[:, k, :],
                            start=False, stop=(k == KH - 1))
                    bias = bibht
                else:
                    bias = biht
                dst = gxc[:, mi, cs] if mi < MC else gxn2[:, mi - MC, cs]
                nc.scalar.activation(dst, ps_gx, AF.Identity,
                                     bias=bias[:, m:m + 1])

            def u_c1(mi, nb, rznc):
                # deferred (noncrit) gate chunk; mi in 0..11 over chunks 12..23
                # ct folds in via ses_rep matmuls; PSUM evac is one ScalarE op;
                # the n-gate elementwise chain runs on GPSIMD (SBUF only).
                m = MC + mi
                cs = slice(nb * NBW, (nb + 1) * NBW)
                ps_nc = psMM.tile([128, NBW], F32, tag="mm")
                if mi < 8:   # r_n / z_n: gx folds into the same PSUM group
                    for k in range(EP // 128):
                        nc.tensor.matmul(
                            out=ps_nc,
                            lhsT=wih_sb[:, k, m * 128:(m + 1) * 128],
                            rhs=embxT[:, k, cs],
                            start=(k == 0), stop=False)
                for k in range(KH):
                    nc.tensor.matmul(
                        out=ps_nc,
                        lhsT=whh_n_sb[:, k, mi * 128:(mi + 1) * 128],
                        rhs=hsT[:, k, cs],
                        start=(mi >= 8 and k == 0), stop=False)
                for k in range(KH):
                    nc.tensor.matmul(
                        out=ps_nc,
                        lhsT=whh_s_sb[:, k, m * 128:(m + 1) * 128],
                        rhs=ses_rep[:, k, :],
                        start=False, stop=(k == KH - 1))
                if mi < 8:
                    nc.scalar.activation(rznc[:, mi, :], ps_nc, AF.Sigmoid,
                                         bias=bibht[:, m:m + 1])
                else:        # n_n: tanh(gxn2 + r*(ps + ct))
                    j = mi - 8
                    ghn_nc = post.tile([128, NBW], F32, tag="ghnnc")
                    nc.scalar.activation(ghn_nc, ps_nc, AF.Identity,
                                         bias=bhht[:, m:m + 1])
                    t1n = post.tile([128, NBW], F32, tag="t1n")
                    nc.gpsimd.tensor_tensor(out=t1n, in0=rznc[:, j, :],
                                            in1=ghn_nc, op=OP.mult)
                    nc.gpsimd.tensor_tensor(out=t1n, in0=t1n,
                                            in1=gxn2[:, j, cs], op=OP.add)
                    ntn = post.tile([128, NBW], F32, tag="ntn")
                    nc.scalar.activation(ntn, t1n, AF.Tanh)
                    # hn_nc = (1-z)*n + z*ses  (all SBUF -> gpsimd)
                    bzn = post.tile([128, NBW], F32, tag="bzn")
                    nc.gpsimd.tensor_scalar(out=bzn, in0=rznc[:, 4 + j, :],
                                            scalar1=-1.0, scalar2=1.0,
                                            op0=OP.mult, op1=OP.add)
                    an = post.tile([128, NBW], F32, tag="an")
                    nc.gpsimd.tensor_tensor(out=an, in0=rznc[:, 4 + j, :],
                                            in1=ses_rep[:, j, :], op=OP.mult)
                    cn = post.tile([128, NBW], F32, tag="cn")
                    nc.gpsimd.tensor_tensor(out=cn, in0=bzn, in1=ntn, op=OP.mult)
                    nc.gpsimd.tensor_tensor(out=hnT_nc[:, j, cs], in0=an,
                                            in1=cn, op=OP.add)

            def u_c2(m, nb):
                pm = min(128, E - m * 128)
                cs = slice(nb * NBW, (nb + 1) * NBW)
                css = slice(BL + nb * NBW, BL + (nb + 1) * NBW)
                ps_o = psMM.tile([128, NBW], F32, tag="mm")
                for k in range(G // 128):
                    rhs = hsT[:, k, css] if k < KH else hnT_nc[:, k - KH, cs]
                    nc.tensor.matmul(
                        out=ps_o[:pm, :],
                        lhsT=w2_sb[:, k, m * 128:m * 128 + pm],
                        rhs=rhs,
                        start=(k == 0), stop=(k == G // 128 - 1))
                nc.vector.scalar_tensor_tensor(
                    out=oT[:pm, m, cs], in0=ps_o[:pm, :],
                    scalar=b2t[:pm, m:m + 1],
                    in1=embxT[:pm, m, cs], op0=OP.add, op1=OP.add)

            def u_wdma(nv):
                nw = min(512, V - nv * 512)
                wchunk = woutp.tile([128, EP // 128, 512], BF16, tag="w")
                nc.sync.dma_start(
                    out=wchunk[:, :, :nw],
                    in_=d_wout[:, nv * 512:nv * 512 + nw].rearrange(
                        "(k p) v -> p k v", p=128))
                return wchunk

            def u_c3(nv, wchunk, mts):
                # logits for one vocab block over the given oT row chunks
                nw = min(512, V - nv * 512)
                tagn = f"l{len(mts)}"
                lsb = lout.tile([128, len(mts), 512], BF16, tag=tagn)
                for i, mt in enumerate(mts):
                    pm = 128 if mt < 6 else NT - 6 * 128
                    ms = slice(mt * 128, mt * 128 + pm)
                    ps_l = psL.tile([128, 512], F32, tag="l")
                    for k in range(EP // 128):
                        nc.tensor.matmul(
                            out=ps_l[:pm, :nw],
                            lhsT=oT[:, k, ms],
                            rhs=wchunk[:, k, :nw],
                            start=(k == 0), stop=(k == EP // 128 - 1))
                    if mt % 2 == 0:
                        nc.vector.tensor_copy(lsb[:pm, i, :nw], ps_l[:pm, :nw])
                    else:
                        nc.scalar.copy(lsb[:pm, i, :nw], ps_l[:pm, :nw])
                nc.sync.dma_start(
                    out=d_out[:, nv * 512:nv * 512 + nw].rearrange(
                        "(m p) v -> p m v", p=128)[:, mts[0]:mts[0] + len(mts), :],
                    in_=lsb[:, :, :nw])

            with loop_cm:
              for _rep in rep_iter:
                xw_holder = [None]
                with nc.named_scope("setup"):
                    sesenc_sb = inp.tile([128, SH // 128, BL], BF16, tag="se")
                    xw = inp.tile([128, 7], I32, tag="xw")
                    xw_holder[0] = xw
                    nc.sync.dma_start(out=sesenc_sb, in_=d_sesenc[:, :].rearrange("(k p) c -> p k c", p=128))
                    nc.sync.dma_start(out=xw, in_=d_xw[:, :])

                    # ses = tanh(W1 @ ses_encT + b1)  -> [H, BL]
                    ps_s = psT.tile([128, KH, BL], F32, tag="tp")
                    for m in range(KH):
                        for k in range(SH // 128):
                            nc.tensor.matmul(
                                out=ps_s[:, m, :],
                                lhsT=w1_sb[:, k, m * 128:(m + 1) * 128],
                                rhs=sesenc_sb[:, k, :],
                                start=(k == 0), stop=(k == SH // 128 - 1))
                    for m in range(KH):
                        nc.scalar.activation(sesT_bf[:, m, :], ps_s[:, m, :], AF.Tanh,
                                             bias=b1t[:, m:m + 1])
                    nc.vector.tensor_copy(hsT[:, :, 0:BL], sesT_bf)
                    for k in range(KH):
                        nc.vector.tensor_copy(
                            ses_rep[:, k, :].rearrange("p (t b) -> p t b", b=BL),
                            sesT_bf[:, k, None, :].broadcast_to(
                                [128, NBW // BL, BL]))

                    # CT (n-crit chunks 8..11 only) = Whh_ses @ sesT + bhh
                    ps_gs = psT.tile([128, 4, BL], F32, tag="tp")
                    for mi, m in enumerate(range(8, MC)):
                        for k in range(KH):
                            nc.tensor.matmul(
                                out=ps_gs[:, mi, :],
                                lhsT=whh_s_sb[:, k, m * 128:(m + 1) * 128],
                                rhs=sesT_bf[:, k, :],
                                start=(k == 0), stop=(k == KH - 1))
                    nc.vector.tensor_tensor(
                        out=ct, in0=ps_gs,
                        in1=bhht[:, 8:MC, None].broadcast_to([128, 4, BL]), op=OP.add)

                    # gather first 4 column chunks; crit gx for nb=0
                    for c in range(4):
                        u_gather(c)
                    for mi in range(MC):
                        u_gx(mi, mi, 0)

                # ---- injection schedule: unit emitted after its step ----
                rznc0 = post.tile([128, 8, NBW], BF16, tag="rznc", bufs=1)
                rznc1 = post.tile([128, 8, NBW], BF16, tag="rznc2", bufs=1)
                injections = {}
                for c in range(4, 7):
                    injections.setdefault(c - 4, []).append(lambda c=c: u_gather(c))
                for mi in range(MC):
                    injections.setdefault(3 + mi, []).append(
                        lambda mi=mi: u_gx(mi, mi, 1))
                for i, (mi, nb) in enumerate([(j, b) for b in range(NB)
                                              for j in range(4)]):
                    injections.setdefault(15 + i, []).append(
                        lambda mi=mi, nb=nb: u_gx(MC + mi, 20 + mi, nb))
                if variant not in ("rec", "mmonly", "nochain", "recbf"):
                  for mi in range(12):
                    injections.setdefault(24 + mi, []).append(
                        lambda mi=mi: u_c1(mi, 0, rznc0))
                  for m in range(3):
                    injections.setdefault(36 + m, []).append(
                        lambda m=m: u_c2(m, 0))
                wchunks = {}
                def mk_wdma(nv):
                    def f():
                        wchunks[nv] = u_wdma(nv)
                    return f
                NVA = 11   # vocab blocks whose mt 0..2 run inside the loop
                if variant == "full":
                  for nv in range(NVA):
                    injections.setdefault(38 + nv, []).append(mk_wdma(nv))
                    injections.setdefault(39 + nv, []).append(
                        lambda nv=nv: u_c3(nv, wchunks[nv], (0, 1, 2)))

                # ---- recurrence ----
                # chunk roles (permuted): 0-3 r_c, 4-7 z_c, 8-11 n_c
                use_bf = (variant == "recbf")
                wsrc = whh8b_sb if use_bf else whh8_sb
                SC = 1.0 if use_bf else 0.0625
                with nc.named_scope("recur"):
                    for t in range(T):
                        ts = slice(t * BL, (t + 1) * BL)
                        ts1 = slice((t + 1) * BL, (t + 2) * BL)
                        ps_zr = psG.tile([128, 8, BL], F32, tag="gzr")
                        ps_n = psG.tile([128, 4, BL], F32, tag="gn")
                        for m in (0, 1, 2, 3, 4, 5, 6, 7, 8, 9, 10, 11):
                            ps = ps_zr[:, m, :] if m < 8 else ps_n[:, m - 8, :]
                            for k in range(KH):
                                nc.tensor.matmul(
                                    out=ps,
                                    lhsT=wsrc[:, k, m * 128:(m + 1) * 128],
                                    rhs=hsT[:, k, ts],
                                    start=(k == 0), stop=(k == KH - 1))
                        if variant == "mmonly":
                            nc.vector.tensor_copy(hsT[:, :, ts1], ps_n)
                            for f in injections.get(t, []):
                                f()
                            continue
                        zrp = step.tile([128, 8, BL], F32, tag="zrp")
                        nc.vector.scalar_tensor_tensor(
                            out=zrp, in0=ps_zr, scalar=SC,
                            in1=gxc[:, 0:8, ts], op0=OP.mult, op1=OP.add)
                        rz = step.tile([128, 8, BL], F32, tag="rz")
                        nc.scalar.activation(rz, zrp, AF.Sigmoid)
                        ghn = step.tile([128, 4, BL], F32, tag="ghn")
                        nc.vector.scalar_tensor_tensor(
                            out=ghn, in0=ps_n, scalar=SC,
                            in1=ct, op0=OP.mult, op1=OP.add)
                        t1 = step.tile([128, 4, BL], F32, tag="t1")
                        nc.vector.tensor_tensor(out=t1, in0=rz[:, 0:4, :],
                                                in1=ghn, op=OP.mult)
                        nc.vector.tensor_tensor(out=t1, in0=t1, in1=gxc[:, 8:MC, ts],
                                                op=OP.add)
                        # a = z*h ; bz = 1-z  (off critical path, run during tanh)
                        a = step.tile([128, 4, BL], F32, tag="a")
                        nc.vector.tensor_tensor(out=a, in0=rz[:, 4:8, :],
                                                in1=hsT[:, :, ts], op=OP.mult)
                        bz = step.tile([128, 4, BL], F32, tag="bz")
                        nc.vector.tensor_scalar(out=bz, in0=rz[:, 4:8, :],
                                                scalar1=-1.0, scalar2=1.0,
                                                op0=OP.mult, op1=OP.add)
                        nt = step.tile([128, 4, BL], F32, tag="nt")
                        nc.scalar.activation(nt, t1, AF.Tanh)
                        c_ = step.tile([128, 4, BL], F32, tag="c")
                        nc.vector.tensor_tensor(out=c_, in0=bz, in1=nt, op=OP.mult)
                        if variant == "nochain":
                            hnx = step.tile([128, 4, BL], BF16, tag="hnx")
                            nc.vector.tensor_tensor(out=hnx, in0=a, in1=c_,
                                                    op=OP.add)
                            nc.vector.tensor_copy(hsT[:, :, ts1], ps_n)
                        else:
                            nc.vector.tensor_tensor(out=hsT[:, :, ts1], in0=a,
                                                    in1=c_, op=OP.add)
                        for f in injections.get(t, []):
                            f()

                # ---- tail: remaining deferred work ----
                with nc.named_scope("tail"):
                  if variant not in ("rec", "mmonly", "nochain", "recbf"):
                    for mi in range(12):
                        u_c1(mi, 1, rznc1)
                    for m in range(3):
                        u_c2(m, 1)
                  if variant == "full":
                    # one-group-ahead wout DMA prefetch so the PE never
                    # waits on the 393KB weight fetch
                    groups = ([(nv, [(3, 4, 5, 6)]) for nv in range(NVA)] +
                              [(nv, [(0, 1, 2), (3, 4, 5, 6)])
                               for nv in range(NVA, NV)])
                    wc_next = u_wdma(groups[0][0])
                    for gi, (nv, mtss) in enumerate(groups):
                        wc = wc_next
                        if gi + 1 < len(groups):
                            wc_next = u_wdma(groups[gi + 1][0])
                        for mts in mtss:
                            u_c3(nv, wc, mts)

                if debug and _rep == 0:
                    dbg = {
                        "dbg_ses": ([128, KH * BL], BF16, sesT_bf),
                        "dbg_ct": ([128, 4 * BL], F32, ct),
                        "dbg_embx": ([128, (EP // 128) * NT], BF16, embxT),
                        "dbg_gxc": ([128, MC * NT], BF16, gxc),
                        "dbg_gxn2": ([128, 4 * NT], BF16, gxn2),
                        "dbg_hs": ([128, KH * (NT + 2 * BL)], BF16, hsT),
                        "dbg_hnnc": ([128, KH * NT], BF16, hnT_nc),
                        "dbg_o": ([128, (EP // 128) * NT], BF16, oT),
                    }
                    for nm, (shp, dt, tl) in dbg.items():
                        dh = nc.dram_tensor(nm, shp, dt, kind="ExternalOutput")
                        nc.sync.dma_start(out=dh[:, :], in_=tl[:, :].rearrange("p a b -> p (a b)"))

    nc.finalize()
    return nc


_PROG_CACHE = {}


def _get_program(reps: int = 1):
    if reps not in _PROG_CACHE:
        _PROG_CACHE[reps] = build_program(reps)
    return _PROG_CACHE[reps]


def _bf(a):
    return np.ascontiguousarray(a).astype(ml_dtypes.bfloat16)


def _prep_shared(inputs):
    emb = np.ascontiguousarray(inputs["emb"], dtype=np.float32)
    Wih = np.asarray(inputs["Wih"], dtype=np.float32)[PERM]
    Whh = np.asarray(inputs["Whh"], dtype=np.float32)[PERM]
    bih = np.asarray(inputs["bih"], dtype=np.float32)[PERM]
    bhh = np.asarray(inputs["bhh"], dtype=np.float32)[PERM]
    W1 = np.asarray(inputs["W1"], dtype=np.float32)
    W2 = np.asarray(inputs["W2"], dtype=np.float32)
    Wout = np.asarray(inputs["Wout"], dtype=np.float32)

    WhhT = Whh.T  # [G, 3G] (gate dim permuted)
    wih_p = np.zeros((EP, G3), np.float32)
    wih_p[:E] = Wih.T
    wout_p = np.zeros((EP, V), np.float32)
    wout_p[:E] = Wout.T
    b2_p = np.zeros(EP, np.float32)
    b2_p[:E] = np.asarray(inputs["b2"], dtype=np.float32)

    return {
        "emb": emb,
        "whh8": np.ascontiguousarray(WhhT[:H, :MCW] * 16.0).astype(
            ml_dtypes.float8_e3m4),
        "whh8b": _bf(WhhT[:H, :MCW]),
        "whh_n": _bf(WhhT[:H, MCW:]),
        "whh_s": _bf(WhhT[H:]),
        "wih": _bf(wih_p),
        "w1": _bf(W1.T),
        "w2": _bf(W2.T),
        "wout": _bf(wout_p),
        "b1t": np.ascontiguousarray(
            np.asarray(inputs["b1"], np.float32).reshape(H // 128, 128).T),
        "biht": np.ascontiguousarray(bih.reshape(M3, 128).T),
        "bibht": np.ascontiguousarray((bih + bhh).reshape(M3, 128).T),
        "bhht": np.ascontiguousarray(bhh.reshape(M3, 128).T),
        "b2t": np.ascontiguousarray(b2_p.reshape(EP // 128, 128).T),
    }


def make_in_maps(inputs):
    shared = _prep_shared(inputs)
    x = np.asarray(inputs["x"]).astype(np.int32)          # [B, T]
    ses = np.asarray(inputs["ses_encoding"], np.float32)[0]  # [B, SH]
    in_maps = []
    for c in range(NCORES):
        bs = slice(c * BL, (c + 1) * BL)
        xf = np.zeros(NTP, np.int32)
        xf[:NT] = x[bs].T.reshape(-1)  # t-major
        m = dict(shared)
        m["xw"] = np.ascontiguousarray(xf.reshape(7, 128).T)
        m["sesenc"] = _bf(ses[bs].T)
        in_maps.append(m)
    return in_maps


def run(inputs, reps: int = 1, **kwargs):
    nc = _get_program(reps)
    in_maps = make_in_maps(inputs)
    res = run_bass_kernel_spmd(nc, in_maps, core_ids=list(range(NCORES)), **kwargs)
    out = np.concatenate(
        [res.results[c]["out"][:NT].astype(np.float32).reshape(T, BL, V)
         .transpose(1, 0, 2) for c in range(NCORES)], axis=0)
    return np.ascontiguousarray(out)


def kernel(**inputs) -> np.ndarray:
    return run(inputs)
